# revision 4
# baseline (speedup 1.0000x reference)
"""Trainium2 Bass kernel for a 2-layer dense GCN block:

    z = x.reshape(B, N, F)                     # B=4, N=8192, F=64
    for i in range(2):
        z = relu((A @ z) @ W_i)                # A: [N, N] dense
    return z

Strategy (8 NeuronCores, SPMD):
  * Shard the output rows (m) of A @ Z across cores: core j owns rows
    [1024*j, 1024*(j+1)).  On the host we hand core j the matching
    column-slice of A^T (contraction dim n on SBUF partitions, contiguous
    DMA) cast to bf16 (16 MiB — resident in SBUF for BOTH layers).
  * Z is a [n, c] matrix with c = b*F + f (256 columns).  Layer matmuls
    compute H^T[c, m] = sum_n Z[n, c] * A^T[n, m] on the tensor engine
    (lhsT = Z tile, rhs = A^T tile, fp32 PSUM accum), n-loop outermost so
    PE consumption tracks the chunked A/Z DMA arrival.
  * Weight apply uses a block-diagonal diag(W_i, W_i) tile so one K=128
    matmul per c-half computes Z[m, c] for two batches at once.
  * Layer 1 runs as TWO m-half passes; each pass's tail (PSUM copy,
    weight apply, relu, z1 store and TWO m-sliced AllGathers) is pinned
    ahead of the next pass's matmuls, so all four inter-layer gathers
    are pipelined on the (pre-warmed) CC stream underneath layer-1/2
    compute instead of being exposed at the layer boundary.
  * Gathered Z1 lands in its own resident SBUF tiles (no reuse hazard
    with Z0), and layer 2's n-loop is ordered by gather arrival.
  * bf16 operands / fp32 accumulation (measured ~0.5% rel-l2 vs the
    fp32 reference).  Final output is fp32 on the host.
"""

import numpy as np
import ml_dtypes

import concourse.mybir as mybir
import concourse.tile as tile
from concourse import bacc
from concourse.bass_utils import run_bass_kernel_spmd

BF16 = ml_dtypes.bfloat16

NCORES = 8
B, N, F, L = 4, 8192, 64, 2
C = B * F                      # 256 columns of the Z matrix
M_CORE = N // NCORES           # 1024 output rows per core
NT = N // 128                  # 64 contraction tiles of 128
MT = M_CORE // 128             # 8 output-row tiles of 128 per core
NG = 4                         # m-sliced inter-layer AllGathers
MPG = MT // NG                 # m-tiles per gather slice (2)

# n-tile chunking of the resident A^T / Z0 loads: small leading chunks so
# the first matmuls start as early as possible, big ones after.
CHUNK_T = [2, 2, 4, 8, 8, 8, 8, 8, 8, 8]
assert sum(CHUNK_T) == NT
KCH = len(CHUNK_T)
CHUNK_T0 = np.cumsum([0] + CHUNK_T).tolist()   # start t of each chunk
T2K = {}
for _k in range(KCH):
    for _tt in range(CHUNK_T[_k]):
        T2K[CHUNK_T0[_k] + _tt] = (_k, _tt)

_CACHED = {}


def _build_program():
    nc = bacc.Bacc("TRN2", target_bir_lowering=False, debug=False,
                   num_devices=NCORES)
    dt = mybir.dt

    at_d = nc.dram_tensor("at", [N, M_CORE], dt.bfloat16, kind="ExternalInput")
    z0_d = nc.dram_tensor("z0", [N, C], dt.bfloat16, kind="ExternalInput")
    w_d = nc.dram_tensor("w", [128, 2 * 128], dt.bfloat16, kind="ExternalInput")
    out_d = nc.dram_tensor("out", [M_CORE, C], dt.bfloat16, kind="ExternalOutput")

    z1_loc = nc.dram_tensor("z1_loc", [M_CORE, C], dt.bfloat16)
    warm_in = nc.dram_tensor("warm_in", [1, 128], dt.bfloat16)
    warm_out = nc.dram_tensor("warm_out", [NCORES, 128], dt.bfloat16)
    z1g = [nc.dram_tensor(f"z1g{g}", [NCORES * MPG * 128, C], dt.bfloat16)
           for g in range(NG)]

    with tile.TileContext(nc) as tc:
        with tc.tile_pool(name="a_res", bufs=1) as a_pool, \
             tc.tile_pool(name="z_res", bufs=1) as z_pool, \
             tc.tile_pool(name="z1_res", bufs=1) as z1_pool, \
             tc.tile_pool(name="wk", bufs=1) as w_pool, \
             tc.tile_pool(name="ht", bufs=2, space="PSUM") as psh_pool, \
             tc.tile_pool(name="pz", bufs=4, space="PSUM") as psz_pool, \
             tc.tile_pool(name="hsb", bufs=2) as hsb_pool, \
             tc.tile_pool(name="zout", bufs=8) as zout_pool:

            # Warm the collective path FIRST so the kernel barrier + CC
            # stream init happen under the initial DMA/compute ramp and the
            # first real gather finds a warm stream.
            nc.gpsimd.dma_start(out=warm_in[:], in_=z0_d[0:1, 0:128])
            nc.gpsimd.collective_compute(
                "AllGather",
                mybir.AluOpType.bypass,
                replica_groups=[list(range(NCORES))],
                ins=[warm_in.ap().opt()],
                outs=[warm_out.ap().opt()],
            )

            w_sb = w_pool.tile([128, 2 * 128], dt.bfloat16, tag="w")
            nc.scalar.dma_start(out=w_sb[:], in_=w_d[:])

            # Resident A^T shard (16 MiB bf16) + Z0 tiles, loaded in chunks,
            # A^T triggers on sync, Z0 on vector (spread the DGE load).
            at_sb = [a_pool.tile([128, CHUNK_T[k] * M_CORE], dt.bfloat16,
                                 tag=f"at{k}", name=f"at_sb{k}")
                     for k in range(KCH)]
            z_sb = [z_pool.tile([128, CHUNK_T[k] * C], dt.bfloat16,
                                tag=f"z{k}", name=f"z_sb{k}")
                    for k in range(KCH)]
            # Resident gathered-Z1 tiles: one per gather slice, 16 n-tiles
            # each ([2048, 256] as [128, 16*C]).
            z1_sb = [z1_pool.tile([128, NCORES * MPG * C], dt.bfloat16,
                                  tag=f"z1s{g}", name=f"z1_sb{g}")
                     for g in range(NG)]

            def free3(tile_ap, inner):
                return tile_ap.rearrange("p (t i) -> p t i", i=inner)

            for k in range(KCH):
                t0, tn = CHUNK_T0[k], CHUNK_T[k]
                z_view = z0_d.ap()[t0 * 128:(t0 + tn) * 128, :] \
                    .rearrange("(t p) c -> p t c", p=128)
                at_view = at_d.ap()[t0 * 128:(t0 + tn) * 128, :] \
                    .rearrange("(t p) m -> p t m", p=128)
                nc.scalar.dma_start(out=free3(z_sb[k][:], C), in_=z_view)
                nc.sync.dma_start(out=free3(at_sb[k][:], M_CORE), in_=at_view)

            def z0_tile(t, ch):
                """lhsT: Z0[n-tile t, c-half ch] -> [128, 128] bf16."""
                k, tt = T2K[t]
                return z_sb[k][:, tt * C + ch * 128: tt * C + ch * 128 + 128]

            def z1_tile(t, ch):
                """lhsT: gathered Z1[n-tile t, c-half ch] -> [128, 128]."""
                cb, r = divmod(t, MT)
                g, i = divmod(r, MPG)
                tt = cb * MPG + i
                base = tt * C + ch * 128
                return z1_sb[g][:, base: base + 128]

            def at_tile(t, mh):
                """rhs: A^T[n-tile t, m-half mh] -> [128, 512] bf16."""
                k, tt = T2K[t]
                return at_sb[k][:, tt * M_CORE + mh * 512: tt * M_CORE + mh * 512 + 512]

            import contextlib

            def layer(li, z_tile, passes, on_pass_done=None):
                # passes: list of (mh_tuple, t_order). Each pass accumulates
                # H^T psum for its m-halves over all 64 n-tiles, then applies
                # weights + relu for those m-tiles; the tail is pinned ahead
                # of the next pass's matmuls so stores/collectives overlap.
                res = [None] * MT
                for pi, (mhs, t_order) in enumerate(passes):
                    h_ps = {}
                    for ch in range(2):
                        for mh in mhs:
                            h_ps[ch, mh] = psh_pool.tile(
                                [128, 512], dt.float32,
                                tag=f"hps{ch}", name=f"hps_{li}_{pi}_{ch}{mh}")
                    h_sb = {(ch, mh): hsb_pool.tile(
                                [128, 512], dt.bfloat16, tag=f"h{ch}",
                                name=f"h_sb_{li}_{pi}_{ch}{mh}")
                            for ch in range(2) for mh in mhs}
                    for ti, t in enumerate(t_order):
                        for ch in range(2):
                            for mh in mhs:
                                nc.tensor.matmul(
                                    h_ps[ch, mh][:],
                                    z_tile(t, ch),
                                    at_tile(t, mh),
                                    start=(ti == 0),
                                    stop=(ti == NT - 1),
                                )
                    prio = (tc.high_priority() if pi + 1 < len(passes)
                            else contextlib.nullcontext())
                    with prio:
                        for ch in range(2):
                            for mh in mhs:
                                nc.vector.tensor_copy(
                                    h_sb[ch, mh][:], h_ps[ch, mh][:])
                        tiles = [i for mh in mhs
                                 for i in range(mh * MT // 2, (mh + 1) * MT // 2)]
                        for i in tiles:
                            mh = i // (MT // 2)
                            io = i - mh * (MT // 2)
                            z_ps = psz_pool.tile([128, C], dt.float32, tag="zps",
                                                 name=f"z_ps_{li}_{i}")
                            for ch in range(2):
                                nc.tensor.matmul(
                                    z_ps[:, ch * 128:(ch + 1) * 128],
                                    h_sb[ch, mh][:, io * 128:(io + 1) * 128],
                                    w_sb[:, li * 128:(li + 1) * 128],
                                    start=True, stop=True,
                                )
                            z_o = zout_pool.tile([128, C], dt.bfloat16,
                                                 tag="zo", name=f"z_o_{li}_{i}")
                            nc.scalar.activation(z_o[:], z_ps[:],
                                                 mybir.ActivationFunctionType.Relu)
                            res[i] = z_o
                        if on_pass_done is not None:
                            on_pass_done(tiles, res)
                return res

            # ---- layer 1: two m-half passes, 2 gathers per pass ----
            def gather_pass(tiles, res):
                for i in tiles:
                    nc.scalar.dma_start(out=z1_loc[i * 128:(i + 1) * 128, :],
                                        in_=res[i][:])
                for g in sorted({i // MPG for i in tiles}):
                    nc.gpsimd.collective_compute(
                        "AllGather",
                        mybir.AluOpType.bypass,
                        replica_groups=[list(range(NCORES))],
                        ins=[z1_loc.ap()[g * MPG * 128:(g + 1) * MPG * 128, :].opt()],
                        outs=[z1g[g].ap().opt()],
                    )
                    # immediately stage the gathered slice into SBUF
                    nc.sync.dma_start(
                        out=free3(z1_sb[g][:], C),
                        in_=z1g[g].ap().rearrange("(t p) c -> p t c", p=128))

            layer(0, z0_tile,
                  [((0,), list(range(NT))), ((1,), list(range(NT)))],
                  on_pass_done=gather_pass)

            # ---- layer 2: n-loop ordered by gather arrival ----
            t2 = [cb * MT + g * MPG + i
                  for g in range(NG) for cb in range(NCORES) for i in range(MPG)]
            z2_tiles = layer(1, z1_tile, [((0, 1), t2)])
            for i in range(MT):
                nc.sync.dma_start(out=out_d[i * 128:(i + 1) * 128, :],
                                  in_=z2_tiles[i][:])

    nc.compile()
    return nc


def _prep_inputs(x, net_params, A):
    a_bf = A.astype(BF16)
    z0 = np.ascontiguousarray(x.transpose(1, 0, 2).reshape(N, C)).astype(BF16)
    w = net_params.astype(np.float32).reshape(L, F, F).astype(BF16)
    # block-diagonal weight tile per layer: diag(W_l, W_l)
    w_sb = np.zeros((128, 2 * 128), dtype=BF16)
    for li in range(L):
        w_sb[0:F, li * 128:li * 128 + F] = w[li]
        w_sb[F:2 * F, li * 128 + F:li * 128 + 2 * F] = w[li]
    in_maps = []
    for j in range(NCORES):
        at_j = np.ascontiguousarray(a_bf[j * M_CORE:(j + 1) * M_CORE, :].T)
        in_maps.append({"at": at_j, "z0": z0, "w": w_sb})
    return in_maps


def kernel(x, t, net_params, A):
    x = np.asarray(x)
    A = np.asarray(A)
    net_params = np.asarray(net_params)

    if "nc" not in _CACHED:
        _CACHED["nc"] = _build_program()
    nc = _CACHED["nc"]

    in_maps = _prep_inputs(x, net_params, A)
    _CACHED["in_maps"] = in_maps
    res = run_bass_kernel_spmd(nc, in_maps, list(range(NCORES)))
    full = np.concatenate([res.results[c]["out"] for c in range(NCORES)],
                          axis=0).astype(np.float32)
    return np.ascontiguousarray(full.reshape(N, B, F).transpose(1, 0, 2))


# revision 7
# speedup vs baseline: 1.0505x; 1.0505x over previous
"""Trainium2 Bass kernel for a 2-layer dense GCN block:

    z = x.reshape(B, N, F)                     # B=4, N=8192, F=64
    for i in range(2):
        z = relu((A @ z) @ W_i)                # A: [N, N] dense
    return z

Strategy (8 NeuronCores, SPMD):
  * Shard the output rows (m) of A @ Z across cores: core j owns rows
    [1024*j, 1024*(j+1)).  On the host we hand core j the matching
    column-slice of A^T (contraction dim n on SBUF partitions, contiguous
    DMA) cast to bf16 (16 MiB — resident in SBUF for BOTH layers).
  * Z is a [n, c] matrix with c = b*F + f (256 columns).  Layer matmuls
    compute H^T[c, m] = sum_n Z[n, c] * A^T[n, m] on the tensor engine
    (lhsT = Z tile, rhs = A^T tile, fp32 PSUM accum), n-loop outermost so
    PE consumption tracks the chunked A/Z DMA arrival.
  * Weight apply uses a block-diagonal diag(W_i, W_i) tile so one K=128
    matmul per c-half computes Z[m, c] for two batches at once.
  * Layer 1 runs as TWO m-half passes; each pass's tail (PSUM copy,
    weight apply, relu, z1 store and TWO m-sliced AllGathers) is pinned
    ahead of the next pass's matmuls, so all four inter-layer gathers
    are pipelined on the (pre-warmed) CC stream underneath layer-1/2
    compute instead of being exposed at the layer boundary.
  * Gathered Z1 lands in its own resident SBUF tiles (no reuse hazard
    with Z0), and layer 2's n-loop is ordered by gather arrival.
  * bf16 operands / fp32 accumulation (measured ~0.5% rel-l2 vs the
    fp32 reference).  Final output is fp32 on the host.
"""

import numpy as np
import ml_dtypes

import concourse.mybir as mybir
import concourse.tile as tile
from concourse import bacc
from concourse.bass_utils import run_bass_kernel_spmd

BF16 = ml_dtypes.bfloat16

NCORES = 8
B, N, F, L = 4, 8192, 64, 2
C = B * F                      # 256 columns of the Z matrix
M_CORE = N // NCORES           # 1024 output rows per core
NT = N // 128                  # 64 contraction tiles of 128
MT = M_CORE // 128             # 8 output-row tiles of 128 per core
NG = 4                         # m-sliced inter-layer AllGathers
MPG = MT // NG                 # m-tiles per gather slice (2)

# n-tile chunking of the resident A^T / Z0 loads: small leading chunks so
# the first matmuls start as early as possible, big ones after.
CHUNK_T = [2, 2, 4, 8, 8, 8, 8, 8, 8, 8]
assert sum(CHUNK_T) == NT
KCH = len(CHUNK_T)
CHUNK_T0 = np.cumsum([0] + CHUNK_T).tolist()   # start t of each chunk
T2K = {}
for _k in range(KCH):
    for _tt in range(CHUNK_T[_k]):
        T2K[CHUNK_T0[_k] + _tt] = (_k, _tt)

_CACHED = {}


def _build_program():
    nc = bacc.Bacc("TRN2", target_bir_lowering=False, debug=False,
                   num_devices=NCORES)
    dt = mybir.dt

    at_d = nc.dram_tensor("at", [N, M_CORE], dt.bfloat16, kind="ExternalInput")
    z0_d = nc.dram_tensor("z0", [N, C], dt.bfloat16, kind="ExternalInput")
    w_d = nc.dram_tensor("w", [128, 2 * 128], dt.bfloat16, kind="ExternalInput")
    out_d = nc.dram_tensor("out", [M_CORE, C], dt.bfloat16, kind="ExternalOutput")

    z1_loc = nc.dram_tensor("z1_loc", [M_CORE, C], dt.bfloat16)
    z1g = [nc.dram_tensor(f"z1g{g}", [NCORES * MPG * 128, C], dt.bfloat16,
                          addr_space="Shared")
           for g in range(NG)]

    with tile.TileContext(nc) as tc:
        with tc.tile_pool(name="a_res", bufs=1) as a_pool, \
             tc.tile_pool(name="z_res", bufs=1) as z_pool, \
             tc.tile_pool(name="z1_res", bufs=1) as z1_pool, \
             tc.tile_pool(name="wk", bufs=1) as w_pool, \
             tc.tile_pool(name="ht", bufs=2, space="PSUM") as psh_pool, \
             tc.tile_pool(name="pz", bufs=4, space="PSUM") as psz_pool, \
             tc.tile_pool(name="hsb", bufs=2) as hsb_pool, \
             tc.tile_pool(name="zout", bufs=8) as zout_pool:

            w_sb = w_pool.tile([128, 2 * 128], dt.bfloat16, tag="w")
            nc.scalar.dma_start(out=w_sb[:], in_=w_d[:])

            # Resident A^T shard (16 MiB bf16) + Z0 tiles, loaded in chunks,
            # A^T triggers on sync, Z0 on vector (spread the DGE load).
            at_sb = [a_pool.tile([128, CHUNK_T[k] * M_CORE], dt.bfloat16,
                                 tag=f"at{k}", name=f"at_sb{k}")
                     for k in range(KCH)]
            z_sb = [z_pool.tile([128, CHUNK_T[k] * C], dt.bfloat16,
                                tag=f"z{k}", name=f"z_sb{k}")
                    for k in range(KCH)]
            # Resident gathered-Z1 tiles: one per gather slice, 16 n-tiles
            # each ([2048, 256] as [128, 16*C]).
            z1_sb = [z1_pool.tile([128, NCORES * MPG * C], dt.bfloat16,
                                  tag=f"z1s{g}", name=f"z1_sb{g}")
                     for g in range(NG)]

            def free3(tile_ap, inner):
                return tile_ap.rearrange("p (t i) -> p t i", i=inner)

            for k in range(KCH):
                t0, tn = CHUNK_T0[k], CHUNK_T[k]
                z_view = z0_d.ap()[t0 * 128:(t0 + tn) * 128, :] \
                    .rearrange("(t p) c -> p t c", p=128)
                at_view = at_d.ap()[t0 * 128:(t0 + tn) * 128, :] \
                    .rearrange("(t p) m -> p t m", p=128)
                nc.gpsimd.dma_start(out=free3(z_sb[k][:], C), in_=z_view)
                # alternate A^T chunk triggers between the two hwdge queues
                at_eng = nc.sync if k % 2 == 0 else nc.scalar
                at_eng.dma_start(out=free3(at_sb[k][:], M_CORE), in_=at_view)

            def z0_tile(t, ch):
                """lhsT: Z0[n-tile t, c-half ch] -> [128, 128] bf16."""
                k, tt = T2K[t]
                return z_sb[k][:, tt * C + ch * 128: tt * C + ch * 128 + 128]

            def z1_tile(t, ch):
                """lhsT: gathered Z1[n-tile t, c-half ch] -> [128, 128]."""
                cb, r = divmod(t, MT)
                g, i = divmod(r, MPG)
                tt = cb * MPG + i
                base = tt * C + ch * 128
                return z1_sb[g][:, base: base + 128]

            def at_tile(t, mh):
                """rhs: A^T[n-tile t, m-half mh] -> [128, 512] bf16."""
                k, tt = T2K[t]
                return at_sb[k][:, tt * M_CORE + mh * 512: tt * M_CORE + mh * 512 + 512]

            import contextlib

            def layer(li, z_tile, passes, on_pass_done=None):
                # passes: list of (mh_tuple, t_order). Each pass accumulates
                # H^T psum for its m-halves over all 64 n-tiles, then applies
                # weights + relu for those m-tiles; the tail is pinned ahead
                # of the next pass's matmuls so stores/collectives overlap.
                res = [None] * MT
                for pi, (mhs, t_order) in enumerate(passes):
                    h_ps = {}
                    for ch in range(2):
                        for mh in mhs:
                            h_ps[ch, mh] = psh_pool.tile(
                                [128, 512], dt.float32,
                                tag=f"hps{ch}", name=f"hps_{li}_{pi}_{ch}{mh}")
                    h_sb = {(ch, mh): hsb_pool.tile(
                                [128, 512], dt.bfloat16, tag=f"h{ch}",
                                name=f"h_sb_{li}_{pi}_{ch}{mh}")
                            for ch in range(2) for mh in mhs}
                    for ti, t in enumerate(t_order):
                        for ch in range(2):
                            for mh in mhs:
                                nc.tensor.matmul(
                                    h_ps[ch, mh][:],
                                    z_tile(t, ch),
                                    at_tile(t, mh),
                                    start=(ti == 0),
                                    stop=(ti == NT - 1),
                                )
                    prio = (tc.high_priority() if pi + 1 < len(passes)
                            else contextlib.nullcontext())
                    with prio:
                        for ch in range(2):
                            for mh in mhs:
                                nc.vector.tensor_copy(
                                    h_sb[ch, mh][:], h_ps[ch, mh][:])
                        tiles = [i for mh in mhs
                                 for i in range(mh * MT // 2, (mh + 1) * MT // 2)]
                        for i in tiles:
                            mh = i // (MT // 2)
                            io = i - mh * (MT // 2)
                            z_ps = psz_pool.tile([128, C], dt.float32, tag="zps",
                                                 name=f"z_ps_{li}_{i}")
                            for ch in range(2):
                                nc.tensor.matmul(
                                    z_ps[:, ch * 128:(ch + 1) * 128],
                                    h_sb[ch, mh][:, io * 128:(io + 1) * 128],
                                    w_sb[:, li * 128:(li + 1) * 128],
                                    start=True, stop=True,
                                )
                            z_o = zout_pool.tile([128, C], dt.bfloat16,
                                                 tag="zo", name=f"z_o_{li}_{i}")
                            nc.scalar.activation(z_o[:], z_ps[:],
                                                 mybir.ActivationFunctionType.Relu)
                            res[i] = z_o
                        if on_pass_done is not None:
                            on_pass_done(tiles, res)
                return res

            # ---- layer 1: two m-half passes, 2 gathers per pass ----
            def gather_pass(tiles, res):
                for i in tiles:
                    nc.scalar.dma_start(out=z1_loc[i * 128:(i + 1) * 128, :],
                                        in_=res[i][:])
                for g in sorted({i // MPG for i in tiles}):
                    nc.gpsimd.collective_compute(
                        "AllGather",
                        mybir.AluOpType.bypass,
                        replica_groups=[list(range(NCORES))],
                        ins=[z1_loc.ap()[g * MPG * 128:(g + 1) * MPG * 128, :].opt()],
                        outs=[z1g[g].ap().opt()],
                    )
                    # immediately stage the gathered slice into SBUF
                    nc.sync.dma_start(
                        out=free3(z1_sb[g][:], C),
                        in_=z1g[g].ap().rearrange("(t p) c -> p t c", p=128))

            layer(0, z0_tile,
                  [((0,), list(range(NT))), ((1,), list(range(NT)))],
                  on_pass_done=gather_pass)

            # ---- layer 2: n-loop ordered by gather arrival ----
            t2 = [cb * MT + g * MPG + i
                  for g in range(NG) for cb in range(NCORES) for i in range(MPG)]
            z2_tiles = layer(1, z1_tile, [((0, 1), t2)])
            for i in range(MT):
                nc.sync.dma_start(out=out_d[i * 128:(i + 1) * 128, :],
                                  in_=z2_tiles[i][:])

    nc.compile()
    return nc


def _prep_inputs(x, net_params, A):
    a_bf = A.astype(BF16)
    z0 = np.ascontiguousarray(x.transpose(1, 0, 2).reshape(N, C)).astype(BF16)
    w = net_params.astype(np.float32).reshape(L, F, F).astype(BF16)
    # block-diagonal weight tile per layer: diag(W_l, W_l)
    w_sb = np.zeros((128, 2 * 128), dtype=BF16)
    for li in range(L):
        w_sb[0:F, li * 128:li * 128 + F] = w[li]
        w_sb[F:2 * F, li * 128 + F:li * 128 + 2 * F] = w[li]
    in_maps = []
    for j in range(NCORES):
        at_j = np.ascontiguousarray(a_bf[j * M_CORE:(j + 1) * M_CORE, :].T)
        in_maps.append({"at": at_j, "z0": z0, "w": w_sb})
    return in_maps


def kernel(x, t, net_params, A):
    x = np.asarray(x)
    A = np.asarray(A)
    net_params = np.asarray(net_params)

    if "nc" not in _CACHED:
        _CACHED["nc"] = _build_program()
    nc = _CACHED["nc"]

    in_maps = _prep_inputs(x, net_params, A)
    _CACHED["in_maps"] = in_maps
    res = run_bass_kernel_spmd(nc, in_maps, list(range(NCORES)))
    full = np.concatenate([res.results[c]["out"] for c in range(NCORES)],
                          axis=0).astype(np.float32)
    return np.ascontiguousarray(full.reshape(N, B, F).transpose(1, 0, 2))


# revision 9
# speedup vs baseline: 1.0930x; 1.0405x over previous
"""Trainium2 Bass kernel for a 2-layer dense GCN block:

    z = x.reshape(B, N, F)                     # B=4, N=8192, F=64
    for i in range(2):
        z = relu((A @ z) @ W_i)                # A: [N, N] dense
    return z

Strategy (8 NeuronCores, SPMD):
  * Shard the output rows (m) of A @ Z across cores: core j owns rows
    [1024*j, 1024*(j+1)).  Core j gets the matching column-slice of A^T
    (contraction dim n on SBUF partitions) cast to bf16: 16 MiB, resident
    in SBUF for BOTH layers, so A is read from HBM exactly once.
  * Z is a [n, c] matrix with c = b*F + f (256 columns).  Layer matmuls
    compute H^T[c, m] = sum_n Z[n, c] * A^T[n, m] (lhsT = Z tile,
    rhs = A^T tile, fp32 PSUM accum).
  * Layer-1 schedule is built around two serial resources measured from
    traces: the initial 20 MiB A/Z load (~60us of DMA) and the single CC
    stream (whose first op — the runtime kernel barrier — only completes
    ~55us in).  n-tiles 0..31 run chunk-major over BOTH m-halves so PE
    consumption matches DMA arrival; n-tiles 32..63 then run as two
    m-half sweeps (their A^T halves are DMA'd mh-0-first so the sweep
    never starves).  Each half's tail (PSUM copy, weight apply via a
    block-diagonal diag(W,W) tile, relu, z1 store, two m-sliced
    AllGathers) fires as soon as that half stops, so all four
    inter-layer gathers pipeline on the CC stream from ~65us on,
    underneath the rest of layer 1 and the start of layer 2.
  * Gathered Z1 is staged into its own resident SBUF tiles (half-slice
    DMAs to cut landing latency) and layer 2's n-loop is ordered by
    gather arrival, with the final 16 n-tiles again split into two
    m-half sweeps so the output tail overlaps the last accumulation.
  * bf16 operands / fp32 accumulation (measured ~0.5% rel-l2 vs the
    fp32 reference).  Final output is fp32 on the host.
"""

import contextlib

import numpy as np
import ml_dtypes

import concourse.mybir as mybir
import concourse.tile as tile
from concourse import bacc
from concourse.bass_utils import run_bass_kernel_spmd

BF16 = ml_dtypes.bfloat16

NCORES = 8
B, N, F, L = 4, 8192, 64, 2
C = B * F                      # 256 columns of the Z matrix
M_CORE = N // NCORES           # 1024 output rows per core
NT = N // 128                  # 64 contraction tiles of 128
MT = M_CORE // 128             # 8 output-row tiles of 128 per core
NG = 4                         # m-sliced inter-layer AllGathers
MPG = MT // NG                 # m-tiles per gather slice (2)

STAG = 32                      # n-tiles covered by the m-half sweeps
CHUNK_T = [2, 2, 4, 8, 8, 8]   # full-m chunks covering t 0..31
assert sum(CHUNK_T) == NT - STAG
KCH = len(CHUNK_T)
CHUNK_T0 = np.cumsum([0] + CHUNK_T).tolist()
T2K = {}
for _k in range(KCH):
    for _tt in range(CHUNK_T[_k]):
        T2K[CHUNK_T0[_k] + _tt] = (_k, _tt)
HCH = STAG // 8                # 8-tile half-m chunks covering t 32..63

_CACHED = {}


def _build_program():
    nc = bacc.Bacc("TRN2", target_bir_lowering=False, debug=False,
                   num_devices=NCORES)
    dt = mybir.dt

    at_d = nc.dram_tensor("at", [N, M_CORE], dt.bfloat16, kind="ExternalInput")
    z0_d = nc.dram_tensor("z0", [N, C], dt.bfloat16, kind="ExternalInput")
    w_d = nc.dram_tensor("w", [128, 2 * 128], dt.bfloat16, kind="ExternalInput")
    out_d = nc.dram_tensor("out", [M_CORE, C], dt.bfloat16, kind="ExternalOutput")

    z1_loc = nc.dram_tensor("z1_loc", [M_CORE, C], dt.bfloat16)
    z1g = [nc.dram_tensor(f"z1g{g}", [NCORES * MPG * 128, C], dt.bfloat16,
                          addr_space="Shared")
           for g in range(NG)]

    with tile.TileContext(nc) as tc:
        with tc.tile_pool(name="a_res", bufs=1) as a_pool, \
             tc.tile_pool(name="z_res", bufs=1) as z_pool, \
             tc.tile_pool(name="z1_res", bufs=1) as z1_pool, \
             tc.tile_pool(name="wk", bufs=1) as w_pool, \
             tc.tile_pool(name="ht", bufs=2, space="PSUM") as psh_pool, \
             tc.tile_pool(name="pz", bufs=4, space="PSUM") as psz_pool, \
             tc.tile_pool(name="hsb", bufs=2) as hsb_pool, \
             tc.tile_pool(name="zout", bufs=8) as zout_pool:

            w_sb = w_pool.tile([128, 2 * 128], dt.bfloat16, tag="w")
            nc.scalar.dma_start(out=w_sb[:], in_=w_d[:])

            # Full-m A^T chunks for t 0..31 (alternate the two hwdge
            # queues), then per-m-half chunks for t 32..63, mh=0 first.
            at_sb = [a_pool.tile([128, CHUNK_T[k] * M_CORE], dt.bfloat16,
                                 tag=f"at{k}", name=f"at_sb{k}")
                     for k in range(KCH)]
            ath_sb = {(mh, j): a_pool.tile([128, 8 * 512], dt.bfloat16,
                                           tag=f"ath{mh}{j}",
                                           name=f"ath_sb{mh}{j}")
                      for mh in range(2) for j in range(HCH)}
            z_sb = [z_pool.tile([128, CHUNK_T[k] * C], dt.bfloat16,
                                tag=f"z{k}", name=f"z_sb{k}")
                    for k in range(KCH)]
            zs_sb = [z_pool.tile([128, 8 * C], dt.bfloat16,
                                 tag=f"zs{j}", name=f"zs_sb{j}")
                     for j in range(HCH)]
            z1_sb = [z1_pool.tile([128, NCORES * MPG * C], dt.bfloat16,
                                  tag=f"z1s{g}", name=f"z1_sb{g}")
                     for g in range(NG)]

            def free3(tile_ap, inner):
                return tile_ap.rearrange("p (t i) -> p t i", i=inner)

            # z0: all 64 tiles on the gpsimd queue (~4 MiB, lands early)
            for k in range(KCH):
                t0, tn = CHUNK_T0[k], CHUNK_T[k]
                nc.gpsimd.dma_start(
                    out=free3(z_sb[k][:], C),
                    in_=z0_d.ap()[t0 * 128:(t0 + tn) * 128, :]
                        .rearrange("(t p) c -> p t c", p=128))
            for j in range(HCH):
                t0 = STAG + 8 * j
                nc.gpsimd.dma_start(
                    out=free3(zs_sb[j][:], C),
                    in_=z0_d.ap()[t0 * 128:(t0 + 8) * 128, :]
                        .rearrange("(t p) c -> p t c", p=128))

            # A^T full chunks: alternate sync/scalar queues
            for k in range(KCH):
                t0, tn = CHUNK_T0[k], CHUNK_T[k]
                eng = nc.sync if k % 2 == 0 else nc.scalar
                eng.dma_start(
                    out=free3(at_sb[k][:], M_CORE),
                    in_=at_d.ap()[t0 * 128:(t0 + tn) * 128, :]
                        .rearrange("(t p) m -> p t m", p=128))
            # A^T half chunks: all mh=0 before any mh=1
            for mh in range(2):
                for j in range(HCH):
                    t0 = STAG + 8 * j
                    eng = nc.sync if j % 2 == 0 else nc.scalar
                    eng.dma_start(
                        out=free3(ath_sb[mh, j][:], 512),
                        in_=at_d.ap()[t0 * 128:(t0 + 8) * 128,
                                      mh * 512:(mh + 1) * 512]
                            .rearrange("(t p) m -> p t m", p=128))

            def z0_tile(t, ch):
                """lhsT: Z0[n-tile t, c-half ch] -> [128, 128] bf16."""
                if t < NT - STAG:
                    k, tt = T2K[t]
                    return z_sb[k][:, tt * C + ch * 128: tt * C + ch * 128 + 128]
                j, tt = divmod(t - STAG, 8)
                return zs_sb[j][:, tt * C + ch * 128: tt * C + ch * 128 + 128]

            def z1_tile(t, ch):
                """lhsT: gathered Z1[n-tile t, c-half ch] -> [128, 128]."""
                cb, r = divmod(t, MT)
                g, i = divmod(r, MPG)
                tt = cb * MPG + i
                base = tt * C + ch * 128
                return z1_sb[g][:, base: base + 128]

            def at_tile(t, mh):
                """rhs: A^T[n-tile t, m-half mh] -> [128, 512] bf16."""
                if t < NT - STAG:
                    k, tt = T2K[t]
                    return at_sb[k][:, tt * M_CORE + mh * 512:
                                    tt * M_CORE + mh * 512 + 512]
                j, tt = divmod(t - STAG, 8)
                return ath_sb[mh, j][:, tt * 512:(tt + 1) * 512]

            def layer(li, z_tile, t_order, on_half_done):
                """One GCN layer: chunk-major accumulation over the first
                NT-16 tiles of t_order... the first len-STAG tiles run
                both m-halves; the last STAG tiles run as two m-half
                sweeps whose tails fire independently."""
                h_ps = {(ch, mh): psh_pool.tile([128, 512], dt.float32,
                                                tag=f"hps{ch}",
                                                name=f"hps_{li}_{ch}{mh}")
                        for ch in range(2) for mh in range(2)}
                h_sb = {(ch, mh): hsb_pool.tile([128, 512], dt.bfloat16,
                                                tag=f"h{ch}",
                                                name=f"h_sb_{li}_{ch}{mh}")
                        for ch in range(2) for mh in range(2)}
                res = [None] * MT
                head, sweep = t_order[:NT - STAG], t_order[NT - STAG:]
                for ti, t in enumerate(head):
                    for ch in range(2):
                        for mh in range(2):
                            nc.tensor.matmul(
                                h_ps[ch, mh][:], z_tile(t, ch), at_tile(t, mh),
                                start=(ti == 0), stop=False)
                for mh in range(2):
                    for si, t in enumerate(sweep):
                        for ch in range(2):
                            nc.tensor.matmul(
                                h_ps[ch, mh][:], z_tile(t, ch), at_tile(t, mh),
                                start=False, stop=(si == STAG - 1))
                    # half tail: copy, weight-apply, relu, then the
                    # caller's store/collective hook — pinned ahead of
                    # whatever is emitted next.
                    last = (li == 1 and mh == 1)
                    prio = contextlib.nullcontext() if last else tc.high_priority()
                    with prio:
                        for ch in range(2):
                            nc.vector.tensor_copy(h_sb[ch, mh][:],
                                                  h_ps[ch, mh][:])
                        tiles = list(range(mh * MT // 2, (mh + 1) * MT // 2))
                        for i in tiles:
                            io = i - mh * (MT // 2)
                            z_ps = psz_pool.tile([128, C], dt.float32,
                                                 tag="zps",
                                                 name=f"z_ps_{li}_{i}")
                            for ch in range(2):
                                nc.tensor.matmul(
                                    z_ps[:, ch * 128:(ch + 1) * 128],
                                    h_sb[ch, mh][:, io * 128:(io + 1) * 128],
                                    w_sb[:, li * 128:(li + 1) * 128],
                                    start=True, stop=True)
                            z_o = zout_pool.tile([128, C], dt.bfloat16,
                                                 tag="zo", name=f"z_o_{li}_{i}")
                            nc.scalar.activation(
                                z_o[:], z_ps[:],
                                mybir.ActivationFunctionType.Relu)
                            res[i] = z_o
                        on_half_done(mh, tiles, res)
                return res

            # ---- layer 1: tails store z1 + fire the m-sliced gathers ----
            def l1_half_done(mh, tiles, res):
                for i in tiles:
                    nc.scalar.dma_start(out=z1_loc[i * 128:(i + 1) * 128, :],
                                        in_=res[i][:])
                for g in sorted({i // MPG for i in tiles}):
                    nc.gpsimd.collective_compute(
                        "AllGather",
                        mybir.AluOpType.bypass,
                        replica_groups=[list(range(NCORES))],
                        ins=[z1_loc.ap()[g * MPG * 128:(g + 1) * MPG * 128, :].opt()],
                        outs=[z1g[g].ap().opt()],
                    )
                    # stage the gathered slice into SBUF in two halves so
                    # layer 2's first consumers see minimum latency
                    rows = NCORES * MPG * 128
                    hw = (rows // 2 // 128) * C          # SBUF cols per half
                    for h in range(2):
                        r0, r1 = h * rows // 2, (h + 1) * rows // 2
                        nc.sync.dma_start(
                            out=free3(z1_sb[g][:, h * hw:(h + 1) * hw], C),
                            in_=z1g[g].ap()[r0:r1, :]
                                .rearrange("(t p) c -> p t c", p=128))

            layer(0, z0_tile, list(range(NT)), l1_half_done)

            # ---- layer 2: n-loop ordered by gather arrival ----
            def l2_half_done(mh, tiles, res):
                for i in tiles:
                    nc.sync.dma_start(out=out_d[i * 128:(i + 1) * 128, :],
                                      in_=res[i][:])

            t2 = [cb * MT + g * MPG + i
                  for g in range(NG) for cb in range(NCORES) for i in range(MPG)]
            layer(1, z1_tile, t2, l2_half_done)

    nc.compile()
    return nc


def _prep_inputs(x, net_params, A):
    a_bf = A.astype(BF16)
    z0 = np.ascontiguousarray(x.transpose(1, 0, 2).reshape(N, C)).astype(BF16)
    w = net_params.astype(np.float32).reshape(L, F, F).astype(BF16)
    # block-diagonal weight tile per layer: diag(W_l, W_l)
    w_sb = np.zeros((128, 2 * 128), dtype=BF16)
    for li in range(L):
        w_sb[0:F, li * 128:li * 128 + F] = w[li]
        w_sb[F:2 * F, li * 128 + F:li * 128 + 2 * F] = w[li]
    in_maps = []
    for j in range(NCORES):
        at_j = np.ascontiguousarray(a_bf[j * M_CORE:(j + 1) * M_CORE, :].T)
        in_maps.append({"at": at_j, "z0": z0, "w": w_sb})
    return in_maps


def kernel(x, t, net_params, A):
    x = np.asarray(x)
    A = np.asarray(A)
    net_params = np.asarray(net_params)

    if "nc" not in _CACHED:
        _CACHED["nc"] = _build_program()
    nc = _CACHED["nc"]

    in_maps = _prep_inputs(x, net_params, A)
    _CACHED["in_maps"] = in_maps
    res = run_bass_kernel_spmd(nc, in_maps, list(range(NCORES)))
    full = np.concatenate([res.results[c]["out"] for c in range(NCORES)],
                          axis=0).astype(np.float32)
    return np.ascontiguousarray(full.reshape(N, B, F).transpose(1, 0, 2))
